# revision 7
# baseline (speedup 1.0000x reference)
"""GCN classifier (2x GCNConv + mean-pool + linear) on 8 Trainium2 NeuronCores.

Strategy:
  - Destination-node sharding: core c owns nodes [6250c, 6250(c+1)).
  - Self-loops appended as explicit edges; edges sorted by destination block.
  - y1 = dinv * (x @ W1) computed replicated on every core (reads x once).
  - Layer aggregation: per 128-edge chunk, indirect-DMA gather of source rows
    + one-hot selection matrix (DVE is_equal) + PE matmul accumulate into PSUM.
  - h1 -> xw2 -> y2 slice computed per dest block; y2 slices exchanged via
    chunked AllGather collectives overlapped with layer-1 aggregation.
  - Mean-pool via selection matmuls into persistent PSUM accumulators,
    AllReduce of per-graph partial sums/counts, final linear on every core.
"""
import numpy as np

import concourse.bacc as bacc
import concourse.bass as bass
import concourse.mybir as mybir
import concourse.tile as tile
from concourse import bass_utils

# problem dims (hardcoded per contract)
N = 50000
E = 600000
IN_CH = 256
HID = 128
NUM_CLASSES = 2
G = 256
NCORES = 8

SLICE = N // NCORES          # 6250 nodes per core
NB = (SLICE + 127) // 128    # 49 dest blocks per core
PB = NB * 128                # 6272 padded rows per core
PADN = PB * NCORES           # 50176 padded table rows
CB = 7                       # dest blocks per allgather chunk
NX = NB // CB                # 7 allgather chunks
XROWS = CB * 128             # 896 rows per core per chunk

dt = mybir.dt

_cache = {}


def _pos1(u):
    """node id -> row in core-major padded table (y1 layout)."""
    return PB * (u // SLICE) + (u % SLICE)


def _pos2(u):
    """node id -> row in chunk-major padded table (y2 allgather layout)."""
    c = u // SLICE
    l = u % SLICE
    b = l // 128
    p = l % 128
    return (b // CB) * (NCORES * XROWS) + c * XROWS + (b % CB) * 128 + p


def _host_prep(x, edge_index, batch):
    x = np.asarray(x, np.float32)
    ei = np.asarray(edge_index)
    batch_np = np.asarray(batch)

    src = np.concatenate([ei[0], np.arange(N, dtype=ei.dtype)]).astype(np.int64)
    dst = np.concatenate([ei[1], np.arange(N, dtype=ei.dtype)]).astype(np.int64)

    deg = np.bincount(dst, minlength=N).astype(np.float32)  # >= 1 (self-loops)

    # per-core edge lists sorted by local dest block
    per_core = []
    counts = np.zeros((NCORES, NB), np.int64)
    for c in range(NCORES):
        m = (dst // SLICE) == c
        es, ed = src[m], dst[m]
        ld = ed - SLICE * c
        order = np.argsort(ld, kind="stable")
        es, ld = es[order], ld[order]
        blocks = ld // 128
        cnt = np.bincount(blocks, minlength=NB)
        per_core.append((es, ld, np.concatenate([[0], np.cumsum(cnt)])))
        counts[c] = cnt

    # uniform per-block chunk counts (max over cores) so the program is SPMD
    K = np.maximum((counts.max(axis=0) + 127) // 128, 1)  # [NB]
    base = np.concatenate([[0], np.cumsum(K)])
    C = int(base[-1])  # total chunk columns

    row1 = np.zeros((NCORES, 128, C), np.int32)
    row2 = np.zeros((NCORES, 128, C), np.int32)
    colloc = np.full((NCORES, 128, C), -1.0, np.float32)
    for c in range(NCORES):
        es, ld, cks = per_core[c]
        for b in range(NB):
            e0, e1 = cks[b], cks[b + 1]
            k = e1 - e0
            slots = int(K[b]) * 128
            s_pad = np.zeros(slots, np.int64)
            l_pad = np.full(slots, -1.0, np.float32)
            s_pad[:k] = es[e0:e1]
            l_pad[:k] = (ld[e0:e1] % 128).astype(np.float32)
            r1 = _pos1(s_pad).astype(np.int32).reshape(int(K[b]), 128).T
            r2 = _pos2(s_pad).astype(np.int32).reshape(int(K[b]), 128).T
            ll = l_pad.reshape(int(K[b]), 128).T
            row1[c, :, base[b] : base[b + 1]] = r1
            row2[c, :, base[b] : base[b + 1]] = r2
            colloc[c, :, base[b] : base[b + 1]] = ll

    # degrees in block layout
    degp_slices = []
    degs_core = []
    batch_core = []
    for c in range(NCORES):
        dpad = np.ones(PB, np.float32)
        dpad[:SLICE] = deg[c * SLICE : (c + 1) * SLICE]
        degp_slices.append(dpad)
        degs_core.append(dpad.reshape(NB, 128).T.copy())
        bpad = np.full(PB, -1.0, np.float32)
        bpad[:SLICE] = batch_np[c * SLICE : (c + 1) * SLICE].astype(np.float32)
        batch_core.append(bpad.reshape(NB, 128).T.copy())
    degp = np.concatenate(degp_slices).reshape(NCORES * NB, 128).T.copy()  # [128, 392]

    # padded transposed x
    xp = np.zeros((PADN, IN_CH), np.float32)
    xp[_pos1(np.arange(N))] = x
    xT = np.ascontiguousarray(xp.T)  # [256, PADN]

    return {
        "xT": xT,
        "degp": degp,
        "degs": degs_core,
        "batch": batch_core,
        "row1": row1,
        "row2": row2,
        "colloc": colloc,
        "K": K,
        "base": base,
        "C": C,
    }


def _build_program(C, K, base):
    nc = bacc.Bacc("TRN2", target_bir_lowering=False, debug=False, num_devices=NCORES)
    f32, i32 = dt.float32, dt.int32

    # ---- I/O ----
    xT_d = nc.dram_tensor("xT", [IN_CH, PADN], f32, kind="ExternalInput")
    w1_d = nc.dram_tensor("W1", [IN_CH, HID], f32, kind="ExternalInput")
    w2_d = nc.dram_tensor("W2", [HID, HID], f32, kind="ExternalInput")
    b1r_d = nc.dram_tensor("b1r", [128, HID], f32, kind="ExternalInput")
    b2r_d = nc.dram_tensor("b2r", [128, HID], f32, kind="ExternalInput")
    linw_d = nc.dram_tensor("linW", [HID, NUM_CLASSES], f32, kind="ExternalInput")
    linbr_d = nc.dram_tensor("linbr", [128, NUM_CLASSES], f32, kind="ExternalInput")
    iota128_d = nc.dram_tensor("iota128", [128, 128], f32, kind="ExternalInput")
    iota256_d = nc.dram_tensor("iota256", [128, G], f32, kind="ExternalInput")
    ident_d = nc.dram_tensor("ident", [128, 128], f32, kind="ExternalInput")
    degp_d = nc.dram_tensor("degp", [128, NCORES * NB], f32, kind="ExternalInput")
    degs_d = nc.dram_tensor("degs", [128, NB], f32, kind="ExternalInput")
    batch_d = nc.dram_tensor("batch", [128, NB], f32, kind="ExternalInput")
    row1_d = nc.dram_tensor("row1", [128, C], i32, kind="ExternalInput")
    row2_d = nc.dram_tensor("row2", [128, C], i32, kind="ExternalInput")
    colloc_d = nc.dram_tensor("colloc", [128, C], f32, kind="ExternalInput")
    out_d = nc.dram_tensor("out", [G, NUM_CLASSES], f32, kind="ExternalOutput")

    # ---- internal DRAM ----
    y1t = nc.dram_tensor("y1t", [PADN, HID], f32, kind="Internal")
    y2slice = nc.dram_tensor("y2slice", [PB, HID], f32, kind="Internal")
    y2t = nc.dram_tensor("y2t", [PADN, HID], f32, kind="Internal", addr_space="Shared")
    pool_in = nc.dram_tensor("pool_in", [G, HID + 1], f32, kind="Internal")
    pool_out = nc.dram_tensor(
        "pool_out", [G, HID + 1], f32, kind="Internal", addr_space="Shared"
    )

    AG_GROUPS = [list(range(NCORES))]

    with tile.TileContext(nc) as tc:
        with (
            tc.tile_pool(name="consts", bufs=1) as cons,
            tc.tile_pool(name="xtiles", bufs=3) as xtl,
            tc.tile_pool(name="work", bufs=3) as work,
            tc.tile_pool(name="msgs", bufs=8) as msgs,
            tc.tile_pool(name="sels", bufs=8) as sels,
            tc.tile_pool(name="psum", bufs=2, space="PSUM") as pst,
            tc.tile_pool(name="psumx", bufs=1, space="PSUM") as psx,
            tc.tile_pool(name="psump", bufs=1, space="PSUM") as psp,
        ):
            # ---- constants ----
            w1a = cons.tile([128, HID], f32, tag="w1a")
            w1b = cons.tile([128, HID], f32, tag="w1b")
            w2 = cons.tile([128, HID], f32, tag="w2")
            b1r = cons.tile([128, HID], f32, tag="b1r")
            b2r = cons.tile([128, HID], f32, tag="b2r")
            linw = cons.tile([HID, NUM_CLASSES], f32, tag="linw")
            linbr = cons.tile([128, NUM_CLASSES], f32, tag="linbr")
            iota128 = cons.tile([128, 128], f32, tag="iota128")
            iota256 = cons.tile([128, G], f32, tag="iota256")
            ident = cons.tile([128, 128], f32, tag="ident")
            degp = cons.tile([128, NCORES * NB], f32, tag="degp")
            degs = cons.tile([128, NB], f32, tag="degs")
            batcht = cons.tile([128, NB], f32, tag="batcht")
            row1 = cons.tile([128, C], i32, tag="row1")
            row2 = cons.tile([128, C], i32, tag="row2")
            colloc = cons.tile([128, C], f32, tag="colloc")
            dinv_g = cons.tile([128, NCORES * NB], f32, tag="dinv_g")
            dinv_s = cons.tile([128, NB], f32, tag="dinv_s")

            nc.sync.dma_start(w1a[:], w1_d[0:128, :])
            nc.sync.dma_start(w1b[:], w1_d[128:256, :])
            nc.sync.dma_start(w2[:], w2_d[:])
            nc.sync.dma_start(b1r[:], b1r_d[:])
            nc.sync.dma_start(b2r[:], b2r_d[:])
            nc.sync.dma_start(linw[:], linw_d[:])
            nc.sync.dma_start(linbr[:], linbr_d[:])
            nc.sync.dma_start(iota128[:], iota128_d[:])
            nc.sync.dma_start(iota256[:], iota256_d[:])
            nc.sync.dma_start(ident[:], ident_d[:])
            nc.sync.dma_start(degp[:], degp_d[:])
            nc.sync.dma_start(degs[:], degs_d[:])
            nc.sync.dma_start(batcht[:], batch_d[:])
            nc.sync.dma_start(row1[:], row1_d[:])
            nc.sync.dma_start(row2[:], row2_d[:])
            nc.sync.dma_start(colloc[:], colloc_d[:])

            # dinv = 1/sqrt(deg)  (Rsqrt activation is banned for accuracy)
            nc.scalar.activation(dinv_g[:], degp[:], mybir.ActivationFunctionType.Sqrt)
            nc.vector.reciprocal(dinv_g[:], dinv_g[:])
            nc.scalar.activation(dinv_s[:], degs[:], mybir.ActivationFunctionType.Sqrt)
            nc.vector.reciprocal(dinv_s[:], dinv_s[:])

            # ---- phase A: y1 = dinv * (x @ W1), full table, replicated ----
            XW = 16  # blocks per x tile
            nblocks_g = NCORES * NB  # 392
            for t0 in range(0, nblocks_g, XW):
                tb = min(XW, nblocks_g - t0)
                xa = xtl.tile([128, XW * 128], f32, tag="xa")
                xb = xtl.tile([128, XW * 128], f32, tag="xb")
                nc.sync.dma_start(
                    xa[:, : tb * 128], xT_d[0:128, t0 * 128 : (t0 + tb) * 128]
                )
                nc.sync.dma_start(
                    xb[:, : tb * 128], xT_d[128:256, t0 * 128 : (t0 + tb) * 128]
                )
                yout = xtl.tile([128, XW * 128], f32, tag="yout")
                for bi in range(tb):
                    g = t0 + bi
                    ps = pst.tile([128, HID], f32, tag="agg", space="PSUM")
                    nc.tensor.matmul(
                        ps[:], xa[:, bi * 128 : (bi + 1) * 128], w1a[:],
                        start=True, stop=False,
                    )
                    nc.tensor.matmul(
                        ps[:], xb[:, bi * 128 : (bi + 1) * 128], w1b[:],
                        start=False, stop=True,
                    )
                    nc.vector.tensor_tensor(
                        out=yout[:, bi * 128 : (bi + 1) * 128],
                        in0=ps[:],
                        in1=dinv_g[:, g : g + 1].to_broadcast([128, 128]),
                        op=mybir.AluOpType.mult,
                    )
                nc.sync.dma_start(
                    y1t[:].rearrange("(g p) f -> p g f", p=128)[:, t0 : t0 + tb, :],
                    yout[:, : tb * 128].rearrange("p (g f) -> p g f", f=128),
                )

            # ---- phases B (layer 1) and C (layer 2) ----
            for layer in (1, 2):
                rowt = row1 if layer == 1 else row2
                table = y1t if layer == 1 else y2t
                brep = b1r if layer == 1 else b2r
                for b in range(NB):
                    nch = int(base[b + 1] - base[b])
                    ps = pst.tile([128, HID], f32, tag="agg", space="PSUM")
                    for j in range(nch):
                        ci = int(base[b]) + j
                        msg = msgs.tile([128, HID], f32, tag="msg")
                        nc.gpsimd.indirect_dma_start(
                            out=msg[:],
                            out_offset=None,
                            in_=table[:],
                            in_offset=bass.IndirectOffsetOnAxis(
                                ap=rowt[:, ci : ci + 1], axis=0
                            ),
                        )
                        S = sels.tile([128, 128], f32, tag="sel")
                        nc.vector.tensor_tensor(
                            out=S[:],
                            in0=colloc[:, ci : ci + 1].to_broadcast([128, 128]),
                            in1=iota128[:],
                            op=mybir.AluOpType.is_equal,
                        )
                        nc.tensor.matmul(
                            ps[:], S[:], msg[:], start=(j == 0), stop=(j == nch - 1)
                        )
                    # epilogue: h = relu(dinv * agg + b)
                    h = work.tile([128, HID], f32, tag="h")
                    nc.vector.tensor_tensor(
                        out=h[:],
                        in0=ps[:],
                        in1=dinv_s[:, b : b + 1].to_broadcast([128, HID]),
                        op=mybir.AluOpType.mult,
                    )
                    nc.vector.tensor_tensor(
                        out=h[:], in0=h[:], in1=brep[:], op=mybir.AluOpType.add
                    )
                    nc.scalar.activation(
                        h[:], h[:], mybir.ActivationFunctionType.Relu
                    )
                    if layer == 1:
                        # xw2 = h1 @ W2 ; y2 = dinv * xw2
                        pt = psx.tile([128, 128], f32, tag="trans", space="PSUM")
                        nc.tensor.transpose(out=pt[:], in_=h[:], identity=ident[:])
                        hT = work.tile([128, 128], f32, tag="hT")
                        nc.vector.tensor_copy(out=hT[:], in_=pt[:])
                        p2 = psx.tile([128, HID], f32, tag="xw2", space="PSUM")
                        nc.tensor.matmul(p2[:], hT[:], w2[:], start=True, stop=True)
                        y2b = work.tile([128, HID], f32, tag="y2b")
                        nc.vector.tensor_tensor(
                            out=y2b[:],
                            in0=p2[:],
                            in1=dinv_s[:, b : b + 1].to_broadcast([128, HID]),
                            op=mybir.AluOpType.mult,
                        )
                        nc.sync.dma_start(y2slice[b * 128 : (b + 1) * 128, :], y2b[:])
                        if (b + 1) % CB == 0:
                            j = (b + 1) // CB - 1
                            nc.gpsimd.collective_compute(
                                "AllGather",
                                mybir.AluOpType.bypass,
                                replica_groups=AG_GROUPS,
                                ins=[y2slice[j * XROWS : (j + 1) * XROWS, :].opt()],
                                outs=[
                                    y2t[
                                        j * NCORES * XROWS : (j + 1) * NCORES * XROWS, :
                                    ].opt()
                                ],
                            )
                    else:
                        # pooling: append ones column, selection matmuls
                        ho = work.tile([128, HID + 1], f32, tag="hones")
                        nc.vector.tensor_copy(out=ho[:, :HID], in_=h[:])
                        nc.vector.memset(ho[:, HID : HID + 1], 1.0)
                        Sp = work.tile([128, G], f32, tag="spool")
                        nc.vector.tensor_tensor(
                            out=Sp[:],
                            in0=batcht[:, b : b + 1].to_broadcast([128, G]),
                            in1=iota256[:],
                            op=mybir.AluOpType.is_equal,
                        )
                        if b == 0:
                            ppA = psp.tile([128, HID + 1], f32, tag="poolA", space="PSUM")
                            ppB = psp.tile([128, HID + 1], f32, tag="poolB", space="PSUM")
                        nc.tensor.matmul(
                            ppA[:], Sp[:, 0:128], ho[:], start=(b == 0), stop=(b == NB - 1)
                        )
                        nc.tensor.matmul(
                            ppB[:], Sp[:, 128:256], ho[:], start=(b == 0), stop=(b == NB - 1)
                        )

            # ---- phase D: reduce partial sums, final linear ----
            sA = work.tile([128, HID + 1], f32, tag="sA")
            sB = work.tile([128, HID + 1], f32, tag="sB")
            nc.vector.tensor_copy(out=sA[:], in_=ppA[:])
            nc.vector.tensor_copy(out=sB[:], in_=ppB[:])
            nc.sync.dma_start(pool_in[0:128, :], sA[:])
            nc.sync.dma_start(pool_in[128:256, :], sB[:])
            nc.gpsimd.collective_compute(
                "AllReduce",
                mybir.AluOpType.add,
                replica_groups=AG_GROUPS,
                ins=[pool_in[:].opt()],
                outs=[pool_out[:].opt()],
            )
            for half in range(2):
                s = work.tile([128, HID + 1], f32, tag="sred")
                nc.sync.dma_start(s[:], pool_out[half * 128 : (half + 1) * 128, :])
                cnt = work.tile([128, 1], f32, tag="cnt")
                nc.vector.tensor_scalar_max(cnt[:], s[:, HID : HID + 1], 1.0)
                rc = work.tile([128, 1], f32, tag="rc")
                nc.vector.reciprocal(rc[:], cnt[:])
                pt = psx.tile([128, 128], f32, tag="trans", space="PSUM")
                nc.tensor.transpose(out=pt[:], in_=s[:, 0:HID], identity=ident[:])
                sT = work.tile([128, 128], f32, tag="sT")
                nc.vector.tensor_copy(out=sT[:], in_=pt[:])
                po = psx.tile([128, NUM_CLASSES], f32, tag="outp", space="PSUM")
                nc.tensor.matmul(po[:], sT[:], linw[:], start=True, stop=True)
                ob = work.tile([128, NUM_CLASSES], f32, tag="ob")
                nc.vector.tensor_tensor(
                    out=ob[:],
                    in0=po[:],
                    in1=rc[:].to_broadcast([128, NUM_CLASSES]),
                    op=mybir.AluOpType.mult,
                )
                nc.vector.tensor_tensor(
                    out=ob[:], in0=ob[:], in1=linbr[:], op=mybir.AluOpType.add
                )
                nc.sync.dma_start(out_d[half * 128 : (half + 1) * 128, :], ob[:])

    nc.compile()
    return nc


def _get_program(prep):
    key = (prep["C"], tuple(prep["K"]))
    if key not in _cache:
        _cache[key] = _build_program(prep["C"], prep["K"], prep["base"])
    return _cache[key]


def _run(x, edge_index, batch, W1, b1, W2, b2, lin_W, lin_b, trace=False):
    prep = _host_prep(x, edge_index, batch)
    nc = _get_program(prep)

    W1 = np.ascontiguousarray(np.asarray(W1, np.float32))
    W2 = np.ascontiguousarray(np.asarray(W2, np.float32))
    b1r = np.tile(np.asarray(b1, np.float32)[None, :], (128, 1))
    b2r = np.tile(np.asarray(b2, np.float32)[None, :], (128, 1))
    linw = np.ascontiguousarray(np.asarray(lin_W, np.float32))
    linbr = np.tile(np.asarray(lin_b, np.float32)[None, :], (128, 1))
    iota128 = np.tile(np.arange(128, dtype=np.float32)[None, :], (128, 1))
    iota256 = np.tile(np.arange(G, dtype=np.float32)[None, :], (128, 1))
    ident = np.eye(128, dtype=np.float32)

    in_maps = []
    for c in range(NCORES):
        in_maps.append(
            {
                "xT": prep["xT"],
                "W1": W1,
                "W2": W2,
                "b1r": b1r,
                "b2r": b2r,
                "linW": linw,
                "linbr": linbr,
                "iota128": iota128,
                "iota256": iota256,
                "ident": ident,
                "degp": prep["degp"],
                "degs": prep["degs"][c],
                "batch": prep["batch"][c],
                "row1": np.ascontiguousarray(prep["row1"][c]),
                "row2": np.ascontiguousarray(prep["row2"][c]),
                "colloc": np.ascontiguousarray(prep["colloc"][c]),
            }
        )

    res = bass_utils.run_bass_kernel_spmd(
        nc, in_maps, core_ids=list(range(NCORES)), trace=trace
    )
    return res.results[0]["out"], res.exec_time_ns


def kernel(x, edge_index, batch, W1, b1, W2, b2, lin_W, lin_b):
    out, _ = _run(x, edge_index, batch, W1, b1, W2, b2, lin_W, lin_b)
    return out


# revision 9
# speedup vs baseline: 1.2237x; 1.2237x over previous
"""GCN classifier (2x GCNConv + mean-pool + linear) on 8 Trainium2 NeuronCores.

Strategy:
  - Destination-node sharding: core c owns nodes [6250c, 6250(c+1)).
  - Self-loops appended as explicit edges; edges sorted by (dest block,
    table half, position).
  - y1 = dinv * (x @ W1) computed replicated on every core (bf16 table).
  - Aggregation: per (dest block, table half), one custom dma_gather pulls all
    source rows (bf16, 256B rows) into chunk layout; per 128-edge chunk a
    one-hot selection matrix (DVE is_equal, bf16) and a PE matmul accumulate
    into f32 PSUM.
  - h1 -> xw2 -> y2 slice per dest block; y2 slices exchanged via chunked
    AllGather collectives overlapped with layer-1 aggregation.
  - Mean-pool via selection matmuls into persistent PSUM accumulators,
    AllReduce of per-graph partial sums/counts, final linear on every core.
"""
import numpy as np

import concourse.bacc as bacc
import concourse.bass as bass
import concourse.mybir as mybir
import concourse.tile as tile
from concourse import bass_utils

# problem dims (hardcoded per contract)
N = 50000
E = 600000
IN_CH = 256
HID = 128
NUM_CLASSES = 2
G = 256
NCORES = 8

SLICE = N // NCORES          # 6250 nodes per core
NB = (SLICE + 127) // 128    # 49 dest blocks per core
PB = NB * 128                # 6272 padded rows per core
PADN = PB * NCORES           # 50176 padded table rows
HALF = PADN // 2             # 25088 rows per gather-table half (int16 range)
CB = 7                       # dest blocks per allgather chunk
NX = NB // CB                # 7 allgather chunks
XROWS = CB * 128             # 896 rows per core per chunk
MAXC = 8                     # dma_gather cap: num_idxs <= 1024

dt = mybir.dt

_cache = {}


def _pos1(u):
    """node id -> row in core-major padded table (y1 layout)."""
    return PB * (u // SLICE) + (u % SLICE)


def _pos2(u):
    """node id -> row in chunk-major padded table (y2 allgather layout)."""
    c = u // SLICE
    l = u % SLICE
    b = l // 128
    p = l % 128
    return (b // CB) * (NCORES * XROWS) + c * XROWS + (b % CB) * 128 + p


def _wrap_idx(flat):
    """edge-slot-ordered positions [n] -> dma_gather wrapped layout [128, n//16]."""
    n = flat.shape[0]
    cols = n // 16
    out = np.empty((128, cols), np.int16)
    block = flat.reshape(cols, 16).T  # [16, cols]
    for g in range(8):
        out[g * 16 : (g + 1) * 16] = block
    return out


def _host_prep(x, edge_index, batch):
    x = np.asarray(x, np.float32)
    ei = np.asarray(edge_index)
    batch_np = np.asarray(batch)

    src = np.concatenate([ei[0], np.arange(N, dtype=ei.dtype)]).astype(np.int64)
    dst = np.concatenate([ei[1], np.arange(N, dtype=ei.dtype)]).astype(np.int64)
    deg = np.bincount(dst, minlength=N).astype(np.float32)  # >= 1 (self-loops)

    # per-core edges with per-layer (block, half) grouping
    layers = {1: _pos1, 2: _pos2}
    ecore = {}
    counts = {l: np.zeros((NCORES, NB, 2), np.int64) for l in layers}
    for c in range(NCORES):
        m = (dst // SLICE) == c
        es, ed = src[m], dst[m]
        ld = ed - SLICE * c
        ecore[c] = {}
        for l, posf in layers.items():
            pos = posf(es)
            half = (pos // HALF).astype(np.int64)
            order = np.lexsort((pos, half, ld // 128))
            p_s, h_s, ld_s = pos[order], half[order], ld[order]
            b_s = ld_s // 128
            for b in range(NB):
                for h in (0, 1):
                    sel = (b_s == b) & (h_s == h)
                    counts[l][c, b, h] = sel.sum()
            ecore[c][l] = (p_s, h_s, b_s, ld_s % 128)

    # SPMD-uniform chunk counts per (layer, block, half)
    K2 = {}
    for l in layers:
        k = (counts[l].max(axis=0) + 127) // 128  # [NB, 2]
        K2[l] = k.astype(np.int64)

    idx_np = {}
    colloc_np = {}
    for l in layers:
        C2 = int(K2[l].sum())
        idx_np[l] = np.zeros((NCORES, 128, C2 * 8), np.int16)
        colloc_np[l] = np.full((NCORES, 128, C2), -1.0, np.float32)
        for c in range(NCORES):
            p_s, h_s, b_s, lp_s = ecore[c][l]
            col = 0
            for b in range(NB):
                for h in (0, 1):
                    K = int(K2[l][b, h])
                    if K == 0:
                        continue
                    sel = (b_s == b) & (h_s == h)
                    k = int(sel.sum())
                    slots = K * 128
                    p_pad = np.zeros(slots, np.int64)
                    c_pad = np.full(slots, -1.0, np.float32)
                    p_pad[:k] = p_s[sel] - h * HALF
                    c_pad[:k] = lp_s[sel]
                    idx_np[l][c, :, col * 8 : (col + K) * 8] = _wrap_idx(
                        p_pad.astype(np.int16)
                    )
                    colloc_np[l][c, :, col : col + K] = c_pad.reshape(K, 128).T
                    col += K
        colloc_np[l] = colloc_np[l].astype(np.float32)

    # degrees in block layout
    degp_slices = []
    degs_core = []
    batch_core = []
    for c in range(NCORES):
        dpad = np.ones(PB, np.float32)
        dpad[:SLICE] = deg[c * SLICE : (c + 1) * SLICE]
        degp_slices.append(dpad)
        degs_core.append(dpad.reshape(NB, 128).T.copy())
        bpad = np.full(PB, -1.0, np.float32)
        bpad[:SLICE] = batch_np[c * SLICE : (c + 1) * SLICE].astype(np.float32)
        batch_core.append(bpad.reshape(NB, 128).T.copy())
    degp = np.concatenate(degp_slices).reshape(NCORES * NB, 128).T.copy()

    # padded transposed x in bf16
    xp = np.zeros((PADN, IN_CH), np.float32)
    xp[_pos1(np.arange(N))] = x
    xT = np.ascontiguousarray(xp.T).astype(np.dtype("bfloat16") if hasattr(np, "bfloat16") else np.float32)

    return {
        "xT": np.ascontiguousarray(xp.T),
        "degp": degp,
        "degs": degs_core,
        "batch": batch_core,
        "idx": idx_np,
        "colloc": colloc_np,
        "K2": K2,
    }


def _build_program(K2):
    nc = bacc.Bacc("TRN2", target_bir_lowering=False, debug=False, num_devices=NCORES)
    f32, bf16, i16 = dt.float32, dt.bfloat16, dt.int16
    C2 = {l: int(K2[l].sum()) for l in (1, 2)}

    # ---- I/O ----
    xT_d = nc.dram_tensor("xT", [IN_CH, PADN], bf16, kind="ExternalInput")
    w1_d = nc.dram_tensor("W1", [IN_CH, HID], bf16, kind="ExternalInput")
    w2_d = nc.dram_tensor("W2", [HID, HID], bf16, kind="ExternalInput")
    b1r_d = nc.dram_tensor("b1r", [128, HID], f32, kind="ExternalInput")
    b2r_d = nc.dram_tensor("b2r", [128, HID], f32, kind="ExternalInput")
    linw_d = nc.dram_tensor("linW", [HID, NUM_CLASSES], bf16, kind="ExternalInput")
    linbr_d = nc.dram_tensor("linbr", [128, NUM_CLASSES], f32, kind="ExternalInput")
    iota128_d = nc.dram_tensor("iota128", [128, 128], bf16, kind="ExternalInput")
    iota256_d = nc.dram_tensor("iota256", [128, G], bf16, kind="ExternalInput")
    identb_d = nc.dram_tensor("identb", [128, 128], bf16, kind="ExternalInput")
    degp_d = nc.dram_tensor("degp", [128, NCORES * NB], f32, kind="ExternalInput")
    degs_d = nc.dram_tensor("degs", [128, NB], f32, kind="ExternalInput")
    batch_d = nc.dram_tensor("batch", [128, NB], bf16, kind="ExternalInput")
    idx1_d = nc.dram_tensor("idx1", [128, C2[1] * 8], i16, kind="ExternalInput")
    idx2_d = nc.dram_tensor("idx2", [128, C2[2] * 8], i16, kind="ExternalInput")
    col1_d = nc.dram_tensor("col1", [128, C2[1]], bf16, kind="ExternalInput")
    col2_d = nc.dram_tensor("col2", [128, C2[2]], bf16, kind="ExternalInput")
    out_d = nc.dram_tensor("out", [G, NUM_CLASSES], f32, kind="ExternalOutput")

    # ---- internal DRAM ----
    y1t = nc.dram_tensor("y1t", [PADN, HID], bf16, kind="Internal")
    y2slice = nc.dram_tensor("y2slice", [PB, HID], bf16, kind="Internal")
    y2t = nc.dram_tensor("y2t", [PADN, HID], bf16, kind="Internal", addr_space="Shared")
    pool_in = nc.dram_tensor("pool_in", [G, HID + 1], f32, kind="Internal")
    pool_out = nc.dram_tensor(
        "pool_out", [G, HID + 1], f32, kind="Internal", addr_space="Shared"
    )

    AG_GROUPS = [list(range(NCORES))]

    with tile.TileContext(nc) as tc:
        with (
            tc.tile_pool(name="consts", bufs=1) as cons,
            tc.tile_pool(name="xtiles", bufs=3) as xtl,
            tc.tile_pool(name="work", bufs=3) as work,
            tc.tile_pool(name="idxt", bufs=4) as idxt,
            tc.tile_pool(name="msgs", bufs=4) as msgs,
            tc.tile_pool(name="sels", bufs=8) as sels,
            tc.tile_pool(name="psum", bufs=2, space="PSUM") as pst,
            tc.tile_pool(name="psumx", bufs=1, space="PSUM") as psx,
            tc.tile_pool(name="psump", bufs=1, space="PSUM") as psp,
        ):
            # ---- constants ----
            w1a = cons.tile([128, HID], bf16, tag="w1a")
            w1b = cons.tile([128, HID], bf16, tag="w1b")
            w2 = cons.tile([128, HID], bf16, tag="w2")
            b1r = cons.tile([128, HID], f32, tag="b1r")
            b2r = cons.tile([128, HID], f32, tag="b2r")
            linw = cons.tile([HID, NUM_CLASSES], bf16, tag="linw")
            linbr = cons.tile([128, NUM_CLASSES], f32, tag="linbr")
            iota128 = cons.tile([128, 128], bf16, tag="iota128")
            iota256 = cons.tile([128, G], bf16, tag="iota256")
            identb = cons.tile([128, 128], bf16, tag="identb")
            degp = cons.tile([128, NCORES * NB], f32, tag="degp")
            degs = cons.tile([128, NB], f32, tag="degs")
            batcht = cons.tile([128, NB], bf16, tag="batcht")
            col1 = cons.tile([128, C2[1]], bf16, tag="col1")
            col2 = cons.tile([128, C2[2]], bf16, tag="col2")
            dinv_g = cons.tile([128, NCORES * NB], f32, tag="dinv_g")
            dinv_s = cons.tile([128, NB], f32, tag="dinv_s")

            for t, d in (
                (w1a, w1_d[0:128, :]), (w1b, w1_d[128:256, :]), (w2, w2_d[:]),
                (b1r, b1r_d[:]), (b2r, b2r_d[:]), (linw, linw_d[:]),
                (linbr, linbr_d[:]), (iota128, iota128_d[:]),
                (iota256, iota256_d[:]), (identb, identb_d[:]),
                (degp, degp_d[:]), (degs, degs_d[:]), (batcht, batch_d[:]),
                (col1, col1_d[:]), (col2, col2_d[:]),
            ):
                nc.sync.dma_start(t[:], d)

            # dinv = 1/sqrt(deg)
            nc.scalar.activation(dinv_g[:], degp[:], mybir.ActivationFunctionType.Sqrt)
            nc.vector.reciprocal(dinv_g[:], dinv_g[:])
            nc.scalar.activation(dinv_s[:], degs[:], mybir.ActivationFunctionType.Sqrt)
            nc.vector.reciprocal(dinv_s[:], dinv_s[:])

            # ---- phase A: y1 = dinv * (x @ W1), full table, replicated ----
            XW = 16
            nblocks_g = NCORES * NB
            for t0 in range(0, nblocks_g, XW):
                tb = min(XW, nblocks_g - t0)
                xa = xtl.tile([128, XW * 128], bf16, tag="xa")
                xb = xtl.tile([128, XW * 128], bf16, tag="xb")
                nc.sync.dma_start(
                    xa[:, : tb * 128], xT_d[0:128, t0 * 128 : (t0 + tb) * 128]
                )
                nc.sync.dma_start(
                    xb[:, : tb * 128], xT_d[128:256, t0 * 128 : (t0 + tb) * 128]
                )
                yout = xtl.tile([128, XW * 128], bf16, tag="yout")
                for bi in range(tb):
                    g = t0 + bi
                    ps = pst.tile([128, HID], f32, tag="agg", space="PSUM")
                    nc.tensor.matmul(
                        ps[:], xa[:, bi * 128 : (bi + 1) * 128], w1a[:],
                        start=True, stop=False,
                    )
                    nc.tensor.matmul(
                        ps[:], xb[:, bi * 128 : (bi + 1) * 128], w1b[:],
                        start=False, stop=True,
                    )
                    nc.vector.tensor_tensor(
                        out=yout[:, bi * 128 : (bi + 1) * 128],
                        in0=ps[:],
                        in1=dinv_g[:, g : g + 1].to_broadcast([128, 128]),
                        op=mybir.AluOpType.mult,
                    )
                nc.sync.dma_start(
                    y1t[:].rearrange("(g p) f -> p g f", p=128)[:, t0 : t0 + tb, :],
                    yout[:, : tb * 128].rearrange("p (g f) -> p g f", f=128),
                )

            # ---- phases B (layer 1) and C (layer 2) ----
            for layer in (1, 2):
                K = K2[layer]
                idx_d = idx1_d if layer == 1 else idx2_d
                coll = col1 if layer == 1 else col2
                table = y1t if layer == 1 else y2t
                brep = b1r if layer == 1 else b2r
                ibase = 0
                cbase = 0
                for b in range(NB):
                    tot = int(K[b, 0] + K[b, 1])
                    jj = 0
                    ps = pst.tile([128, HID], f32, tag="agg", space="PSUM")
                    for h in (0, 1):
                        Kh = int(K[b, h])
                        if Kh == 0:
                            continue
                        it = idxt.tile([128, MAXC * 8], i16, tag="it")
                        nc.sync.dma_start(
                            it[:, : Kh * 8], idx_d[:, ibase : ibase + Kh * 8]
                        )
                        mt = msgs.tile([128, MAXC * 128], bf16, tag="msg")
                        nc.gpsimd.dma_gather(
                            out_ap=mt[:, : Kh * 128].rearrange(
                                "p (c e) -> p c e", e=HID
                            ),
                            in_ap=table[h * HALF : (h + 1) * HALF, :],
                            idxs_ap=it[:, : Kh * 8],
                            num_idxs=Kh * 128,
                            num_idxs_reg=Kh * 128,
                            elem_size=HID,
                        )
                        for j in range(Kh):
                            S = sels.tile([128, 128], bf16, tag="sel")
                            nc.vector.tensor_tensor(
                                out=S[:],
                                in0=coll[:, cbase + j : cbase + j + 1].to_broadcast(
                                    [128, 128]
                                ),
                                in1=iota128[:],
                                op=mybir.AluOpType.is_equal,
                            )
                            nc.tensor.matmul(
                                ps[:],
                                S[:],
                                mt[:, j * 128 : (j + 1) * 128],
                                start=(jj == 0),
                                stop=(jj == tot - 1),
                            )
                            jj += 1
                        ibase += Kh * 8
                        cbase += Kh
                    # epilogue: h = relu(dinv * agg + b)
                    hf = work.tile([128, HID], f32, tag="hf")
                    nc.vector.tensor_tensor(
                        out=hf[:],
                        in0=ps[:],
                        in1=dinv_s[:, b : b + 1].to_broadcast([128, HID]),
                        op=mybir.AluOpType.mult,
                    )
                    nc.vector.tensor_tensor(
                        out=hf[:], in0=hf[:], in1=brep[:], op=mybir.AluOpType.add
                    )
                    hb = work.tile([128, HID], bf16, tag="hb")
                    nc.scalar.activation(
                        hb[:], hf[:], mybir.ActivationFunctionType.Relu
                    )
                    if layer == 1:
                        # xw2 = h1 @ W2 ; y2 = dinv * xw2
                        pt = psx.tile([128, 128], bf16, tag="trans", space="PSUM")
                        nc.tensor.transpose(out=pt[:], in_=hb[:], identity=identb[:])
                        hT = work.tile([128, 128], bf16, tag="hT")
                        nc.vector.tensor_copy(out=hT[:], in_=pt[:])
                        p2 = psx.tile([128, HID], f32, tag="xw2", space="PSUM")
                        nc.tensor.matmul(p2[:], hT[:], w2[:], start=True, stop=True)
                        y2b = work.tile([128, HID], bf16, tag="y2b")
                        nc.vector.tensor_tensor(
                            out=y2b[:],
                            in0=p2[:],
                            in1=dinv_s[:, b : b + 1].to_broadcast([128, HID]),
                            op=mybir.AluOpType.mult,
                        )
                        nc.sync.dma_start(y2slice[b * 128 : (b + 1) * 128, :], y2b[:])
                        if (b + 1) % CB == 0:
                            j = (b + 1) // CB - 1
                            nc.gpsimd.collective_compute(
                                "AllGather",
                                mybir.AluOpType.bypass,
                                replica_groups=AG_GROUPS,
                                ins=[y2slice[j * XROWS : (j + 1) * XROWS, :].opt()],
                                outs=[
                                    y2t[
                                        j * NCORES * XROWS : (j + 1) * NCORES * XROWS, :
                                    ].opt()
                                ],
                            )
                    else:
                        # pooling: append ones column, selection matmuls
                        ho = work.tile([128, HID + 1], bf16, tag="hones")
                        nc.vector.tensor_copy(out=ho[:, :HID], in_=hb[:])
                        nc.vector.memset(ho[:, HID : HID + 1], 1.0)
                        Sp = work.tile([128, G], bf16, tag="spool")
                        nc.vector.tensor_tensor(
                            out=Sp[:],
                            in0=batcht[:, b : b + 1].to_broadcast([128, G]),
                            in1=iota256[:],
                            op=mybir.AluOpType.is_equal,
                        )
                        if b == 0:
                            ppA = psp.tile([128, HID + 1], f32, tag="poolA", space="PSUM")
                            ppB = psp.tile([128, HID + 1], f32, tag="poolB", space="PSUM")
                        nc.tensor.matmul(
                            ppA[:], Sp[:, 0:128], ho[:], start=(b == 0), stop=(b == NB - 1)
                        )
                        nc.tensor.matmul(
                            ppB[:], Sp[:, 128:256], ho[:], start=(b == 0), stop=(b == NB - 1)
                        )

            # ---- phase D: reduce partial sums, final linear ----
            sA = work.tile([128, HID + 1], f32, tag="sA")
            sB = work.tile([128, HID + 1], f32, tag="sB")
            nc.vector.tensor_copy(out=sA[:], in_=ppA[:])
            nc.vector.tensor_copy(out=sB[:], in_=ppB[:])
            nc.sync.dma_start(pool_in[0:128, :], sA[:])
            nc.sync.dma_start(pool_in[128:256, :], sB[:])
            nc.gpsimd.collective_compute(
                "AllReduce",
                mybir.AluOpType.add,
                replica_groups=AG_GROUPS,
                ins=[pool_in[:].opt()],
                outs=[pool_out[:].opt()],
            )
            for half in range(2):
                s = work.tile([128, HID + 1], f32, tag="sred")
                nc.sync.dma_start(s[:], pool_out[half * 128 : (half + 1) * 128, :])
                cnt = work.tile([128, 1], f32, tag="cnt")
                nc.vector.tensor_scalar_max(cnt[:], s[:, HID : HID + 1], 1.0)
                rc = work.tile([128, 1], f32, tag="rc")
                nc.vector.reciprocal(rc[:], cnt[:])
                sbt = work.tile([128, HID], bf16, tag="sbt")
                nc.vector.tensor_copy(out=sbt[:], in_=s[:, 0:HID])
                pt = psx.tile([128, 128], bf16, tag="trans", space="PSUM")
                nc.tensor.transpose(out=pt[:], in_=sbt[:], identity=identb[:])
                sT = work.tile([128, 128], bf16, tag="sT")
                nc.vector.tensor_copy(out=sT[:], in_=pt[:])
                po = psx.tile([128, NUM_CLASSES], f32, tag="outp", space="PSUM")
                nc.tensor.matmul(po[:], sT[:], linw[:], start=True, stop=True)
                ob = work.tile([128, NUM_CLASSES], f32, tag="ob")
                nc.vector.tensor_tensor(
                    out=ob[:],
                    in0=po[:],
                    in1=rc[:].to_broadcast([128, NUM_CLASSES]),
                    op=mybir.AluOpType.mult,
                )
                nc.vector.tensor_tensor(
                    out=ob[:], in0=ob[:], in1=linbr[:], op=mybir.AluOpType.add
                )
                nc.sync.dma_start(out_d[half * 128 : (half + 1) * 128, :], ob[:])

    nc.compile()
    return nc


def _get_program(prep):
    key = tuple(tuple(map(tuple, prep["K2"][l])) for l in (1, 2))
    if key not in _cache:
        _cache[key] = _build_program(prep["K2"])
    return _cache[key]


def _to_bf16(a):
    import ml_dtypes

    return np.asarray(a, np.float32).astype(ml_dtypes.bfloat16)


def _run(x, edge_index, batch, W1, b1, W2, b2, lin_W, lin_b, trace=False):
    prep = _host_prep(x, edge_index, batch)
    nc = _get_program(prep)

    b1r = np.tile(np.asarray(b1, np.float32)[None, :], (128, 1))
    b2r = np.tile(np.asarray(b2, np.float32)[None, :], (128, 1))
    linbr = np.tile(np.asarray(lin_b, np.float32)[None, :], (128, 1))
    iota128 = _to_bf16(np.tile(np.arange(128, dtype=np.float32)[None, :], (128, 1)))
    iota256 = _to_bf16(np.tile(np.arange(G, dtype=np.float32)[None, :], (128, 1)))
    identb = _to_bf16(np.eye(128, dtype=np.float32))
    xTb = _to_bf16(prep["xT"])
    W1b = _to_bf16(W1)
    W2b = _to_bf16(W2)
    linwb = _to_bf16(lin_W)

    in_maps = []
    for c in range(NCORES):
        in_maps.append(
            {
                "xT": xTb,
                "W1": W1b,
                "W2": W2b,
                "b1r": b1r,
                "b2r": b2r,
                "linW": linwb,
                "linbr": linbr,
                "iota128": iota128,
                "iota256": iota256,
                "identb": identb,
                "degp": prep["degp"],
                "degs": prep["degs"][c],
                "batch": _to_bf16(prep["batch"][c]),
                "idx1": np.ascontiguousarray(prep["idx"][1][c]),
                "idx2": np.ascontiguousarray(prep["idx"][2][c]),
                "col1": _to_bf16(prep["colloc"][1][c]),
                "col2": _to_bf16(prep["colloc"][2][c]),
            }
        )

    res = bass_utils.run_bass_kernel_spmd(
        nc, in_maps, core_ids=list(range(NCORES)), trace=trace
    )
    return res.results[0]["out"], res.exec_time_ns


def kernel(x, edge_index, batch, W1, b1, W2, b2, lin_W, lin_b):
    out, _ = _run(x, edge_index, batch, W1, b1, W2, b2, lin_W, lin_b)
    return out
